# revision 5
# baseline (speedup 1.0000x reference)
"""Trainium2 Bass kernel for the MLPConstructor2 adjacency problem.

Computes, per batch b (one NeuronCore each, 8-way data parallel over B):
    adj[i, j] = tanh(relu(x1_i @ w1 + x2_j @ w2 + b))
for the four (spatial/temporal) quadrants of a (2560, 2560) output.

The output is an outer broadcast-sum of per-row and per-column scalar
vectors, so the kernel is HBM-write bound (26.2 MB/core). Design:

- All eight dot-product vectors (row side a_*, col side c_*) are computed on
  VectorE as mul(+broadcast weight)/reduce in the natural (t p) layout.
  Quadrant biases are folded into the col-side stats.
- Column vectors must appear replicated across all 128 partitions for the
  ScalarE bias-add; that broadcast is done with pure DMA: a transposing
  scatter-store of the [128, T] stat tile to a DRAM scratch (8 KB), then a
  partition-step-0 broadcast-load back as [128, 2560]. No PE, no PSUM --
  fp32 matmuls on the PE are 4 cycles/row and cold-clocked, which dominated
  the v1 setup.
- Main loop per 128-row output tile: 2 ScalarE tanh activations (per-quadrant
  per-partition row bias), 1 VectorE relu in place, one 1.31 MB contiguous
  store. Stores alternate between the Sync and Scalar HWDGE rings.
"""

import numpy as np
from contextlib import ExitStack

import concourse.bacc as bacc
import concourse.mybir as mybir
import concourse.tile as tile
from concourse.bass_utils import run_bass_kernel_spmd

B, N, T, D = 8, 2048, 512, 32
W = N + T            # 2560
NT, TT = N // 128, T // 128   # 16, 4 row-tiles
F32 = mybir.dt.float32
QUADS = ("ss", "st", "ts", "tt")


def _emit(tc, sp, tm, ws, scr, adj):
    nc = tc.nc
    AF = mybir.ActivationFunctionType
    OP = mybir.AluOpType
    with ExitStack() as ctx:
        const = ctx.enter_context(tc.tile_pool(name="const", bufs=1))
        outp = ctx.enter_context(tc.tile_pool(name="outp", bufs=8))

        # ---- stage inputs -------------------------------------------------
        # x tiles in (t p) d layout: row t*128+p lives at [p, t*D : (t+1)*D]
        x_sp = const.tile([128, NT * D], F32)
        nc.sync.dma_start(x_sp[:], sp.rearrange("(t p) d -> p t d", p=128))
        x_tm = const.tile([128, TT * D], F32)
        nc.sync.dma_start(x_tm[:], tm.rearrange("(t p) d -> p t d", p=128))

        # broadcast weights straight from DRAM with step-0 partition APs.
        # layout per side: [128, 4, D] slots =
        #   spatial: [w_ss1, w_st1, w_ss2, w_ts2]; temporal: [w_ts1, w_tt1, w_st2, w_tt2]
        wb_sp = const.tile([128, 4 * D], F32)
        wb_tm = const.tile([128, 4 * D], F32)
        for i, (nm, half) in enumerate([("ss", 0), ("st", 0), ("ss", 1), ("ts", 1)]):
            src = ws[f"w_{nm}"][half * D : (half + 1) * D]
            nc.sync.dma_start(
                wb_sp[:, i * D : (i + 1) * D], src.unsqueeze(0).broadcast_to((128, D))
            )
        for i, (nm, half) in enumerate([("ts", 0), ("tt", 0), ("st", 1), ("tt", 1)]):
            src = ws[f"w_{nm}"][half * D : (half + 1) * D]
            nc.sync.dma_start(
                wb_tm[:, i * D : (i + 1) * D], src.unsqueeze(0).broadcast_to((128, D))
            )
        bb = const.tile([128, 4], F32)   # b_ss, b_st, b_ts, b_tt broadcast
        for j, nm in enumerate(QUADS):
            nc.sync.dma_start(
                bb[:, j : j + 1], ws[f"b_{nm}"].unsqueeze(0).broadcast_to((128, 1))
            )

        # ---- stats on VectorE: prod = x (bcast over 4 slots) * wb, reduce D
        def stats(x, wbt, nt, name):
            # out [128, 4, nt]: slot s, tile t -> dot(x[row], w_slot) in (t p) layout
            prod = const.tile(
                [128, 4 * nt * D], F32, name=f"prod_{name}", tag=f"prod_{name}"
            )
            p4 = prod[:].rearrange("p (s t d) -> p s t d", s=4, t=nt)
            x4 = x[:].rearrange("p (t d) -> p t d", t=nt).unsqueeze(1).broadcast_to(
                (128, 4, nt, D)
            )
            w4 = wbt[:].rearrange("p (s d) -> p s d", s=4).unsqueeze(2).broadcast_to(
                (128, 4, nt, D)
            )
            nc.vector.tensor_tensor(p4, x4, w4, OP.mult)
            st = const.tile([128, 4 * nt], F32, name=f"stat_{name}", tag=f"stat_{name}")
            nc.vector.tensor_reduce(
                st[:].rearrange("p (s t) -> p s t", s=4), p4,
                axis=mybir.AxisListType.X, op=OP.add,
            )
            return st

        st_sp = stats(x_sp, wb_sp, NT, "sp")   # [a_ss, a_st, c_ss, c_ts] x 16
        st_tm = stats(x_tm, wb_tm, TT, "tm")   # [a_ts, a_tt, c_st, c_tt] x 4

        # fold quadrant biases into the col-side stats
        # c_ss += b_ss ; c_ts += b_ts ; c_st += b_st ; c_tt += b_tt
        nc.vector.tensor_scalar_add(
            st_sp[:, 2 * NT : 3 * NT], st_sp[:, 2 * NT : 3 * NT], bb[:, 0:1]
        )
        nc.vector.tensor_scalar_add(
            st_sp[:, 3 * NT : 4 * NT], st_sp[:, 3 * NT : 4 * NT], bb[:, 2:3]
        )
        nc.vector.tensor_scalar_add(
            st_tm[:, 2 * TT : 3 * TT], st_tm[:, 2 * TT : 3 * TT], bb[:, 1:2]
        )
        nc.vector.tensor_scalar_add(
            st_tm[:, 3 * TT : 4 * TT], st_tm[:, 3 * TT : 4 * TT], bb[:, 3:4]
        )

        # ---- column vectors -> DRAM scratch (transposing scatter-store) ---
        # scr_sp layout: [c_ss | c_st] (2560,) ; scr_tm: [c_ts | c_tt]
        # element (p, t) of a stat slot lands at position t*128+p.
        nc.sync.dma_start(
            scr["sp"][0:N].rearrange("(t p) -> p t", p=128), st_sp[:, 2 * NT : 3 * NT]
        )
        nc.sync.dma_start(
            scr["sp"][N:W].rearrange("(t p) -> p t", p=128), st_tm[:, 2 * TT : 3 * TT]
        )
        nc.sync.dma_start(
            scr["tm"][0:N].rearrange("(t p) -> p t", p=128), st_sp[:, 3 * NT : 4 * NT]
        )
        nc.sync.dma_start(
            scr["tm"][N:W].rearrange("(t p) -> p t", p=128), st_tm[:, 3 * TT : 4 * TT]
        )

        # ---- broadcast-load the col tiles [128, 2560] ---------------------
        col_sp = const.tile([128, W], F32)
        nc.sync.dma_start(col_sp[:], scr["sp"].unsqueeze(0).broadcast_to((128, W)))
        col_tm = const.tile([128, W], F32)
        nc.sync.dma_start(col_tm[:], scr["tm"].unsqueeze(0).broadcast_to((128, W)))

        # ---- main loop: 20 output row-tiles of [128, 2560] ----------------
        def row_block(k, row0, col, st, nt, t):
            ot = outp.tile([128, W], F32, name=f"ot{k}", tag="ot")
            nc.scalar.activation(
                ot[:, 0:N], col[:, 0:N], AF.Tanh, bias=st[:, t : t + 1]
            )
            nc.scalar.activation(
                ot[:, N:W], col[:, N:W], AF.Tanh, bias=st[:, nt + t : nt + t + 1]
            )
            nc.vector.tensor_scalar_max(ot[:], ot[:], 0.0)
            eng = nc.sync if k % 2 == 0 else nc.scalar
            eng.dma_start(adj[row0 : row0 + 128, :], ot[:])

        for t in range(NT):
            row_block(t, 128 * t, col_sp, st_sp, NT, t)
        for t in range(TT):
            row_block(NT + t, N + 128 * t, col_tm, st_tm, TT, t)


def build_nc(num_devices=8):
    nc = bacc.Bacc(
        "TRN2",
        target_bir_lowering=False,
        debug=False,
        enable_asserts=True,
        num_devices=num_devices,
    )
    sp = nc.dram_tensor("spatial_nodes", (N, D), F32, kind="ExternalInput").ap()
    tm = nc.dram_tensor("temporal_nodes", (T, D), F32, kind="ExternalInput").ap()
    ws = {}
    for nm in QUADS:
        ws[f"w_{nm}"] = nc.dram_tensor(f"w_{nm}", (2 * D,), F32, kind="ExternalInput").ap()
        ws[f"b_{nm}"] = nc.dram_tensor(f"b_{nm}", (1,), F32, kind="ExternalInput").ap()
    scr = {
        "sp": nc.dram_tensor("scr_sp", (W,), F32, kind="Internal").ap(),
        "tm": nc.dram_tensor("scr_tm", (W,), F32, kind="Internal").ap(),
    }
    adj = nc.dram_tensor("adj", (W, W), F32, kind="ExternalOutput").ap()

    with tile.TileContext(nc) as tc:
        _emit(tc, sp, tm, ws, scr, adj)
    nc.compile()
    return nc


def make_in_maps(inputs):
    in_maps = []
    for b in range(B):
        m = {
            "spatial_nodes": np.ascontiguousarray(inputs["spatial_nodes"][b], np.float32),
            "temporal_nodes": np.ascontiguousarray(inputs["temporal_nodes"][b], np.float32),
        }
        for nm in QUADS:
            m[f"w_{nm}"] = np.ascontiguousarray(inputs[f"w_{nm}"], np.float32)
            m[f"b_{nm}"] = np.ascontiguousarray(inputs[f"b_{nm}"], np.float32)
        in_maps.append(m)
    return in_maps


_NC = {}


def run(inputs, trace=False):
    if 8 not in _NC:
        _NC[8] = build_nc(8)
    res = run_bass_kernel_spmd(
        _NC[8], make_in_maps(inputs), core_ids=list(range(B)), trace=trace
    )
    out = np.stack([res.results[i]["adj"] for i in range(B)], axis=0)
    return out, res


def kernel(**inputs) -> np.ndarray:
    out, _ = run(inputs, trace=False)
    return out


# revision 6
# speedup vs baseline: 1.0836x; 1.0836x over previous
"""Trainium2 Bass kernel for the MLPConstructor2 adjacency problem.

Computes, per batch b (one NeuronCore each, 8-way data parallel over B):
    adj[i, j] = tanh(relu(x1_i @ w1 + x2_j @ w2 + b))
for the four (spatial/temporal) quadrants of a (2560, 2560) output.

The output is an outer broadcast-sum of per-row and per-column scalar
vectors, so the kernel is HBM-write bound (26.2 MB/core). Design:

- x is staged twice, in (t p) layout for the row-side stats (so each
  128-row output tile's biases land on partitions directly) and in (p t)
  layout for the col-side stats (so the stat tile streams out to a DRAM
  scratch contiguously in row order -- no transpose anywhere).
- All eight dot-product vectors are mul(+step-0-broadcast weight)/reduce
  on VectorE; quadrant biases are folded into the col-side stats.
- The column vectors are replicated across partitions with a single
  partition-step-0 DMA broadcast-load of the scratch: pure DMA broadcast.
  No PE, no PSUM (fp32 PE matmuls are 4 cycles/row and cold-clocked).
- Main loop per 128-row output tile: 2 ScalarE tanh activations
  (per-quadrant per-partition row bias), 1 VectorE relu in place, one
  1.31 MB contiguous store, alternating Sync (HWDGE) / GpSimd (SWDGE)
  rings so two DMA queues drain in parallel.
"""

import numpy as np
from contextlib import ExitStack

import concourse.bacc as bacc
import concourse.mybir as mybir
import concourse.tile as tile
from concourse.bass_utils import run_bass_kernel_spmd

B, N, T, D = 8, 2048, 512, 32
W = N + T            # 2560
NT, TT = N // 128, T // 128   # 16, 4 row-tiles
F32 = mybir.dt.float32
QUADS = ("ss", "st", "ts", "tt")


def _emit(tc, sp, tm, ws, scr, adj):
    nc = tc.nc
    AF = mybir.ActivationFunctionType
    OP = mybir.AluOpType
    with ExitStack() as ctx:
        const = ctx.enter_context(tc.tile_pool(name="const", bufs=1))
        outp = ctx.enter_context(tc.tile_pool(name="outp", bufs=8))

        # ---- stage inputs, col-stat layout first (critical path) ----------
        # (p t): row p*nt+t at [p, t*D:(t+1)*D] -- contiguous 2KB per partition
        x_tm_pt = const.tile([128, TT * D], F32)
        nc.sync.dma_start(x_tm_pt[:], tm.rearrange("(p t) d -> p t d", p=128))
        x_sp_pt = const.tile([128, NT * D], F32)
        nc.sync.dma_start(x_sp_pt[:], sp.rearrange("(p t) d -> p t d", p=128))
        # (t p): row t*128+p at [p, t*D:(t+1)*D] -- for row-side bias tiles
        x_sp_tp = const.tile([128, NT * D], F32)
        nc.sync.dma_start(x_sp_tp[:], sp.rearrange("(t p) d -> p t d", p=128))
        x_tm_tp = const.tile([128, TT * D], F32)
        nc.sync.dma_start(x_tm_tp[:], tm.rearrange("(t p) d -> p t d", p=128))

        # broadcast weights straight from DRAM with step-0 partition APs.
        # col-side pairs: wc_sp = [w_ss2, w_ts2], wc_tm = [w_st2, w_tt2]
        # row-side pairs: wr_sp = [w_ss1, w_st1], wr_tm = [w_ts1, w_tt1]
        def wload(name, spec):
            t = const.tile([128, 2 * D], F32, name=name, tag=name)
            for i, (nm, half) in enumerate(spec):
                src = ws[f"w_{nm}"][half * D : (half + 1) * D]
                nc.sync.dma_start(
                    t[:, i * D : (i + 1) * D], src.unsqueeze(0).broadcast_to((128, D))
                )
            return t

        wc_sp = wload("wc_sp", [("ss", 1), ("ts", 1)])
        wc_tm = wload("wc_tm", [("st", 1), ("tt", 1)])
        wr_sp = wload("wr_sp", [("ss", 0), ("st", 0)])
        wr_tm = wload("wr_tm", [("ts", 0), ("tt", 0)])
        bb = const.tile([128, 4], F32)   # b_ss, b_st, b_ts, b_tt broadcast
        for j, nm in enumerate(QUADS):
            nc.sync.dma_start(
                bb[:, j : j + 1], ws[f"b_{nm}"].unsqueeze(0).broadcast_to((128, 1))
            )

        # ---- stats on VectorE: 2-slot mul + reduce over D -----------------
        def stats(x, wpair, nt, name):
            prod = const.tile(
                [128, 2 * nt * D], F32, name=f"prod_{name}", tag=f"prod_{name}"
            )
            p4 = prod[:].rearrange("p (s t d) -> p s t d", s=2, t=nt)
            x4 = x[:].rearrange("p (t d) -> p t d", t=nt).unsqueeze(1).broadcast_to(
                (128, 2, nt, D)
            )
            w4 = wpair[:].rearrange("p (s d) -> p s d", s=2).unsqueeze(2).broadcast_to(
                (128, 2, nt, D)
            )
            nc.vector.tensor_tensor(p4, x4, w4, OP.mult)
            st = const.tile([128, 2 * nt], F32, name=f"stat_{name}", tag=f"stat_{name}")
            nc.vector.tensor_reduce(
                st[:].rearrange("p (s t) -> p s t", s=2), p4,
                axis=mybir.AxisListType.X, op=OP.add,
            )
            return st

        # col stats first: (p t) layout, slots [c_ss, c_ts] / [c_st, c_tt]
        c_tm = stats(x_tm_pt, wc_tm, TT, "c_tm")
        c_sp = stats(x_sp_pt, wc_sp, NT, "c_sp")
        # fold quadrant biases into col stats (c_xy += b_xy)
        nc.vector.tensor_scalar_add(c_sp[:, 0:NT], c_sp[:, 0:NT], bb[:, 0:1])
        nc.vector.tensor_scalar_add(c_tm[:, 0:TT], c_tm[:, 0:TT], bb[:, 1:2])
        nc.vector.tensor_scalar_add(c_sp[:, NT : 2 * NT], c_sp[:, NT : 2 * NT], bb[:, 2:3])
        nc.vector.tensor_scalar_add(c_tm[:, TT : 2 * TT], c_tm[:, TT : 2 * TT], bb[:, 3:4])

        # contiguous stores into the scratch: (p t) traversal == row order
        nc.sync.dma_start(scr["sp"][0:N], c_sp[:, 0:NT])        # c_ss
        nc.sync.dma_start(scr["sp"][N:W], c_tm[:, 0:TT])        # c_st
        nc.sync.dma_start(scr["tm"][0:N], c_sp[:, NT : 2 * NT])  # c_ts
        nc.sync.dma_start(scr["tm"][N:W], c_tm[:, TT : 2 * TT])  # c_tt

        # broadcast-load the col tiles [128, 2560] (split halves so the big
        # half starts as soon as its scratch half lands)
        col_sp = const.tile([128, W], F32)
        nc.sync.dma_start(
            col_sp[:, 0:N], scr["sp"][0:N].unsqueeze(0).broadcast_to((128, N))
        )
        nc.sync.dma_start(
            col_sp[:, N:W], scr["sp"][N:W].unsqueeze(0).broadcast_to((128, T))
        )
        col_tm = const.tile([128, W], F32)
        nc.sync.dma_start(
            col_tm[:, 0:N], scr["tm"][0:N].unsqueeze(0).broadcast_to((128, N))
        )
        nc.sync.dma_start(
            col_tm[:, N:W], scr["tm"][N:W].unsqueeze(0).broadcast_to((128, T))
        )

        # row stats: (t p) layout, slots [a_ss, a_st] / [a_ts, a_tt]
        r_sp = stats(x_sp_tp, wr_sp, NT, "r_sp")
        r_tm = stats(x_tm_tp, wr_tm, TT, "r_tm")

        # ---- main loop: 20 output row-tiles of [128, 2560] ----------------
        def row_block(k, row0, col, st, nt, t):
            ot = outp.tile([128, W], F32, name=f"ot{k}", tag="ot")
            nc.scalar.activation(
                ot[:, 0:N], col[:, 0:N], AF.Tanh, bias=st[:, t : t + 1]
            )
            nc.scalar.activation(
                ot[:, N:W], col[:, N:W], AF.Tanh, bias=st[:, nt + t : nt + t + 1]
            )
            nc.vector.tensor_scalar_max(ot[:], ot[:], 0.0)
            eng = nc.sync if k % 2 == 0 else nc.gpsimd
            eng.dma_start(adj[row0 : row0 + 128, :], ot[:])

        for t in range(NT):
            row_block(t, 128 * t, col_sp, r_sp, NT, t)
        for t in range(TT):
            row_block(NT + t, N + 128 * t, col_tm, r_tm, TT, t)


def build_nc(num_devices=8):
    nc = bacc.Bacc(
        "TRN2",
        target_bir_lowering=False,
        debug=False,
        enable_asserts=True,
        num_devices=num_devices,
    )
    sp = nc.dram_tensor("spatial_nodes", (N, D), F32, kind="ExternalInput").ap()
    tm = nc.dram_tensor("temporal_nodes", (T, D), F32, kind="ExternalInput").ap()
    ws = {}
    for nm in QUADS:
        ws[f"w_{nm}"] = nc.dram_tensor(f"w_{nm}", (2 * D,), F32, kind="ExternalInput").ap()
        ws[f"b_{nm}"] = nc.dram_tensor(f"b_{nm}", (1,), F32, kind="ExternalInput").ap()
    scr = {
        "sp": nc.dram_tensor("scr_sp", (W,), F32, kind="Internal").ap(),
        "tm": nc.dram_tensor("scr_tm", (W,), F32, kind="Internal").ap(),
    }
    adj = nc.dram_tensor("adj", (W, W), F32, kind="ExternalOutput").ap()

    with tile.TileContext(nc) as tc:
        _emit(tc, sp, tm, ws, scr, adj)
    nc.compile()
    return nc


def make_in_maps(inputs):
    in_maps = []
    for b in range(B):
        m = {
            "spatial_nodes": np.ascontiguousarray(inputs["spatial_nodes"][b], np.float32),
            "temporal_nodes": np.ascontiguousarray(inputs["temporal_nodes"][b], np.float32),
        }
        for nm in QUADS:
            m[f"w_{nm}"] = np.ascontiguousarray(inputs[f"w_{nm}"], np.float32)
            m[f"b_{nm}"] = np.ascontiguousarray(inputs[f"b_{nm}"], np.float32)
        in_maps.append(m)
    return in_maps


_NC = {}


def run(inputs, trace=False):
    if 8 not in _NC:
        _NC[8] = build_nc(8)
    res = run_bass_kernel_spmd(
        _NC[8], make_in_maps(inputs), core_ids=list(range(B)), trace=trace
    )
    out = np.stack([res.results[i]["adj"] for i in range(B)], axis=0)
    return out, res


def kernel(**inputs) -> np.ndarray:
    out, _ = run(inputs, trace=False)
    return out


# revision 7
# speedup vs baseline: 1.3297x; 1.2271x over previous
"""Trainium2 Bass kernel for the MLPConstructor2 adjacency problem.

Computes, per batch b (one NeuronCore each, 8-way data parallel over B):
    adj[i, j] = tanh(relu(x1_i @ w1 + x2_j @ w2 + b))
for the four (spatial/temporal) quadrants of a (2560, 2560) output.

The output is an outer broadcast-sum of per-row and per-column scalar
vectors, so the kernel is HBM-write bound (26.2 MB/core). Design:

- x is staged twice, in (t p) layout for the row-side stats (so each
  128-row output tile's biases land on partitions directly) and in (p t)
  layout for the col-side stats (so the stat tile streams out to a DRAM
  scratch contiguously in row order -- no transpose anywhere).
- All eight dot-product vectors are mul(+step-0-broadcast weight)/reduce
  on VectorE; quadrant biases are folded into the col-side stats.
- The column vectors are replicated across partitions with a single
  partition-step-0 DMA broadcast-load of the scratch: pure DMA broadcast.
  No PE, no PSUM (fp32 PE matmuls are 4 cycles/row and cold-clocked).
- Main loop per 128-row output tile: 2 ScalarE tanh activations
  (per-quadrant per-partition row bias), 1 VectorE relu in place, one
  1.31 MB contiguous store, alternating Sync (HWDGE) / GpSimd (SWDGE)
  rings so two DMA queues drain in parallel.
"""

import numpy as np
from contextlib import ExitStack

import concourse.bacc as bacc
import concourse.mybir as mybir
import concourse.tile as tile
from concourse.bass_utils import run_bass_kernel_spmd

B, N, T, D = 8, 2048, 512, 32
W = N + T            # 2560
NT, TT = N // 128, T // 128   # 16, 4 row-tiles
F32 = mybir.dt.float32
QUADS = ("ss", "st", "ts", "tt")


def _emit(tc, sp, tm, ws, scr, adj):
    nc = tc.nc
    AF = mybir.ActivationFunctionType
    OP = mybir.AluOpType
    with ExitStack() as ctx:
        const = ctx.enter_context(tc.tile_pool(name="const", bufs=1))
        outp = ctx.enter_context(tc.tile_pool(name="outp", bufs=8))

        # ---- stage inputs, col-stat layout first (critical path) ----------
        # (p t): row p*nt+t at [p, t*D:(t+1)*D] -- contiguous 2KB per partition
        x_tm_pt = const.tile([128, TT * D], F32)
        nc.sync.dma_start(x_tm_pt[:], tm.rearrange("(p t) d -> p t d", p=128))
        x_sp_pt = const.tile([128, NT * D], F32)
        nc.sync.dma_start(x_sp_pt[:], sp.rearrange("(p t) d -> p t d", p=128))
        # (t p): row t*128+p at [p, t*D:(t+1)*D] -- for row-side bias tiles
        x_sp_tp = const.tile([128, NT * D], F32)
        nc.sync.dma_start(x_sp_tp[:], sp.rearrange("(t p) d -> p t d", p=128))
        x_tm_tp = const.tile([128, TT * D], F32)
        nc.sync.dma_start(x_tm_tp[:], tm.rearrange("(t p) d -> p t d", p=128))

        # broadcast weights straight from DRAM with step-0 partition APs.
        # col-side pairs: wc_sp = [w_ss2, w_ts2], wc_tm = [w_st2, w_tt2]
        # row-side pairs: wr_sp = [w_ss1, w_st1], wr_tm = [w_ts1, w_tt1]
        def wload(name, spec):
            t = const.tile([128, 2 * D], F32, name=name, tag=name)
            for i, (nm, half) in enumerate(spec):
                src = ws[f"w_{nm}"][half * D : (half + 1) * D]
                nc.scalar.dma_start(
                    t[:, i * D : (i + 1) * D], src.unsqueeze(0).broadcast_to((128, D))
                )
            return t

        wc_sp = wload("wc_sp", [("ss", 1), ("ts", 1)])
        wc_tm = wload("wc_tm", [("st", 1), ("tt", 1)])
        wr_sp = wload("wr_sp", [("ss", 0), ("st", 0)])
        wr_tm = wload("wr_tm", [("ts", 0), ("tt", 0)])
        bb = const.tile([128, 4], F32)   # b_ss, b_st, b_ts, b_tt broadcast
        for j, nm in enumerate(QUADS):
            nc.scalar.dma_start(
                bb[:, j : j + 1], ws[f"b_{nm}"].unsqueeze(0).broadcast_to((128, 1))
            )

        # ---- stats on VectorE: 2-slot mul + reduce over D -----------------
        def stats(x, wpair, nt, name):
            prod = const.tile(
                [128, 2 * nt * D], F32, name=f"prod_{name}", tag=f"prod_{name}"
            )
            p4 = prod[:].rearrange("p (s t d) -> p s t d", s=2, t=nt)
            x4 = x[:].rearrange("p (t d) -> p t d", t=nt).unsqueeze(1).broadcast_to(
                (128, 2, nt, D)
            )
            w4 = wpair[:].rearrange("p (s d) -> p s d", s=2).unsqueeze(2).broadcast_to(
                (128, 2, nt, D)
            )
            nc.vector.tensor_tensor(p4, x4, w4, OP.mult)
            st = const.tile([128, 2 * nt], F32, name=f"stat_{name}", tag=f"stat_{name}")
            nc.vector.tensor_reduce(
                st[:].rearrange("p (s t) -> p s t", s=2), p4,
                axis=mybir.AxisListType.X, op=OP.add,
            )
            return st

        # col stats first: (p t) layout, slots [c_ss, c_ts] / [c_st, c_tt]
        c_tm = stats(x_tm_pt, wc_tm, TT, "c_tm")
        c_sp = stats(x_sp_pt, wc_sp, NT, "c_sp")
        # fold quadrant biases into col stats (c_xy += b_xy)
        nc.vector.tensor_scalar_add(c_sp[:, 0:NT], c_sp[:, 0:NT], bb[:, 0:1])
        nc.vector.tensor_scalar_add(c_tm[:, 0:TT], c_tm[:, 0:TT], bb[:, 1:2])
        nc.vector.tensor_scalar_add(c_sp[:, NT : 2 * NT], c_sp[:, NT : 2 * NT], bb[:, 2:3])
        nc.vector.tensor_scalar_add(c_tm[:, TT : 2 * TT], c_tm[:, TT : 2 * TT], bb[:, 3:4])

        # contiguous stores into the scratch: (p t) traversal == row order
        nc.sync.dma_start(scr["sp"][0:N], c_sp[:, 0:NT])        # c_ss
        nc.sync.dma_start(scr["sp"][N:W], c_tm[:, 0:TT])        # c_st
        nc.sync.dma_start(scr["tm"][0:N], c_sp[:, NT : 2 * NT])  # c_ts
        nc.sync.dma_start(scr["tm"][N:W], c_tm[:, TT : 2 * TT])  # c_tt

        # broadcast-load the col tiles [128, 2560] (split halves so the big
        # half starts as soon as its scratch half lands)
        col_sp = const.tile([128, W], F32)
        nc.sync.dma_start(
            col_sp[:, 0:N], scr["sp"][0:N].unsqueeze(0).broadcast_to((128, N))
        )
        nc.sync.dma_start(
            col_sp[:, N:W], scr["sp"][N:W].unsqueeze(0).broadcast_to((128, T))
        )
        col_tm = const.tile([128, W], F32)
        nc.sync.dma_start(
            col_tm[:, 0:N], scr["tm"][0:N].unsqueeze(0).broadcast_to((128, N))
        )
        nc.sync.dma_start(
            col_tm[:, N:W], scr["tm"][N:W].unsqueeze(0).broadcast_to((128, T))
        )

        # row stats: (t p) layout, slots [a_ss, a_st] / [a_ts, a_tt]
        r_sp = stats(x_sp_tp, wr_sp, NT, "r_sp")
        r_tm = stats(x_tm_tp, wr_tm, TT, "r_tm")

        # ---- main loop: 20 output row-tiles of [128, 2560] ----------------
        def row_block(k, row0, col, st, nt, t):
            ot = outp.tile([128, W], F32, name=f"ot{k}", tag="ot")
            nc.scalar.activation(
                ot[:, 0:N], col[:, 0:N], AF.Tanh, bias=st[:, t : t + 1]
            )
            nc.scalar.activation(
                ot[:, N:W], col[:, N:W], AF.Tanh, bias=st[:, nt + t : nt + t + 1]
            )
            nc.vector.tensor_scalar_max(ot[:], ot[:], 0.0)
            nc.sync.dma_start(adj[row0 : row0 + 128, :], ot[:])

        for t in range(NT):
            row_block(t, 128 * t, col_sp, r_sp, NT, t)
        for t in range(TT):
            row_block(NT + t, N + 128 * t, col_tm, r_tm, TT, t)


def build_nc(num_devices=8):
    nc = bacc.Bacc(
        "TRN2",
        target_bir_lowering=False,
        debug=False,
        enable_asserts=True,
        num_devices=num_devices,
    )
    sp = nc.dram_tensor("spatial_nodes", (N, D), F32, kind="ExternalInput").ap()
    tm = nc.dram_tensor("temporal_nodes", (T, D), F32, kind="ExternalInput").ap()
    ws = {}
    for nm in QUADS:
        ws[f"w_{nm}"] = nc.dram_tensor(f"w_{nm}", (2 * D,), F32, kind="ExternalInput").ap()
        ws[f"b_{nm}"] = nc.dram_tensor(f"b_{nm}", (1,), F32, kind="ExternalInput").ap()
    scr = {
        "sp": nc.dram_tensor("scr_sp", (W,), F32, kind="Internal").ap(),
        "tm": nc.dram_tensor("scr_tm", (W,), F32, kind="Internal").ap(),
    }
    adj = nc.dram_tensor("adj", (W, W), F32, kind="ExternalOutput").ap()

    with tile.TileContext(nc) as tc:
        _emit(tc, sp, tm, ws, scr, adj)
    nc.compile()
    return nc


def make_in_maps(inputs):
    in_maps = []
    for b in range(B):
        m = {
            "spatial_nodes": np.ascontiguousarray(inputs["spatial_nodes"][b], np.float32),
            "temporal_nodes": np.ascontiguousarray(inputs["temporal_nodes"][b], np.float32),
        }
        for nm in QUADS:
            m[f"w_{nm}"] = np.ascontiguousarray(inputs[f"w_{nm}"], np.float32)
            m[f"b_{nm}"] = np.ascontiguousarray(inputs[f"b_{nm}"], np.float32)
        in_maps.append(m)
    return in_maps


_NC = {}


def run(inputs, trace=False):
    if 8 not in _NC:
        _NC[8] = build_nc(8)
    res = run_bass_kernel_spmd(
        _NC[8], make_in_maps(inputs), core_ids=list(range(B)), trace=trace
    )
    out = np.stack([res.results[i]["adj"] for i in range(B)], axis=0)
    return out, res


def kernel(**inputs) -> np.ndarray:
    out, _ = run(inputs, trace=False)
    return out
